# revision 5
# baseline (speedup 1.0000x reference)
"""BiLSTM-CRF Trainium2 kernel.

Strategy: 2-way direction parallelism via SPMD (same program, different data).
Core 0 runs the forward LSTM on x; core 1 runs the backward LSTM on reversed x.
Each core: input-projection GEMM (P = Wih @ x + b) streamed to DRAM, sequential
LSTM recurrence with Whh stationary on the tensor engine (gates partition-major
[128,16]) consuming P in 64-step DMA blocks, then a partial feats GEMM
(hs @ Wout_half.T) returned transposed [48, 2048].
Host: embedding gather, feats combine, Viterbi scan + backtrace (tiny, serial).
"""
import sys

sys.path.insert(0, "/opt/trn_rl_repo")

import numpy as np
import concourse.bass as bass
import concourse.bacc as bacc
from concourse import mybir, tile
from concourse.bass_utils import run_bass_kernel_spmd

F32 = mybir.dt.float32
S, E, H, G, T = 2048, 512, 512, 2048, 48
BLK = 64
START, END = T - 2, T - 1
NEG = -10000.0

_CACHE = {}


def _build():
    nc = bacc.Bacc("TRN2", target_bir_lowering=False, debug=False)

    x_t = nc.dram_tensor("x_t", [E, S], F32, kind="ExternalInput")
    wih_t = nc.dram_tensor("wih_t", [E, G], F32, kind="ExternalInput")
    whh_t = nc.dram_tensor("whh_t", [H, G], F32, kind="ExternalInput")
    bias_pm = nc.dram_tensor("bias_pm", [128, 16], F32, kind="ExternalInput")
    h0_pm = nc.dram_tensor("h0_pm", [128, 4], F32, kind="ExternalInput")
    c0_pm = nc.dram_tensor("c0_pm", [128, 4], F32, kind="ExternalInput")
    wout_t = nc.dram_tensor("wout_t", [H, T], F32, kind="ExternalInput")
    feats_t = nc.dram_tensor("feats_t", [T, S], F32, kind="ExternalOutput")
    # P[p, m, t] = proj[z=128m+p, t]; hs[p, t, j] = h_t[128j+p]
    P_dram = nc.dram_tensor("P_dram", [128, 16, S], F32)
    hs_dram = nc.dram_tensor("hs_dram", [128, S, 4], F32)

    Sig = mybir.ActivationFunctionType.Sigmoid
    Tanh = mybir.ActivationFunctionType.Tanh
    ADD = mybir.AluOpType.add
    MULT = mybir.AluOpType.mult

    with tile.TileContext(nc) as tc:
        with (
            tc.tile_pool(name="const", bufs=1) as cpool,
            tc.tile_pool(name="xin", bufs=2) as xpool,
            tc.tile_pool(name="bounce", bufs=4) as bpool,
            tc.tile_pool(name="state", bufs=1) as spool,
            tc.tile_pool(name="psz", bufs=1, space=bass.MemorySpace.PSUM) as zpool,
            tc.tile_pool(name="psg", bufs=2, space=bass.MemorySpace.PSUM) as gpool,
            tc.tile_pool(name="psf", bufs=2, space=bass.MemorySpace.PSUM) as fpool,
        ):
            wih = cpool.tile([128, 4, G], F32)   # [e%128, e//128, g]
            whh = cpool.tile([128, 4, G], F32)   # [h%128, h//128, g]
            wout = cpool.tile([128, 4, T], F32)  # [h%128, h//128, tag]
            bias = cpool.tile([128, 16], F32)
            feats_sb = cpool.tile([48, S], F32)
            h_cur = spool.tile([128, 4], F32)
            c_cur = spool.tile([128, 4], F32)
            P_slot = spool.tile([128, 16, BLK], F32)
            hs_blk = spool.tile([128, BLK, 4], F32)
            zp = zpool.tile([128, 16], F32, tag="zp")
            zs = spool.tile([128, 16], F32)
            sig_if = spool.tile([128, 8], F32)
            tanh_g = spool.tile([128, 4], F32)
            sig_o = spool.tile([128, 4], F32)
            t2 = spool.tile([128, 4], F32)
            c_tmp = spool.tile([128, 4], F32)
            tanh_c = spool.tile([128, 4], F32)

            for j in range(4):
                nc.sync.dma_start(wih[:, j, :], wih_t[128 * j:128 * (j + 1), :])
                nc.sync.dma_start(whh[:, j, :], whh_t[128 * j:128 * (j + 1), :])
                nc.sync.dma_start(wout[:, j, :], wout_t[128 * j:128 * (j + 1), :])
            nc.sync.dma_start(bias[:], bias_pm[:])
            nc.sync.dma_start(h_cur[:], h0_pm[:])
            nc.sync.dma_start(c_cur[:], c0_pm[:])

            # ---- phase 1: P = Wih @ x + b  -> P_dram ----
            for ntile in range(S // 512):
                t0 = ntile * 512
                xa = xpool.tile([128, 4, 512], F32, tag="xa")
                for j in range(4):
                    nc.sync.dma_start(xa[:, j, :], x_t[128 * j:128 * (j + 1), t0:t0 + 512])
                for m in range(16):
                    pg = gpool.tile([128, 512], F32, tag="pg")
                    for j in range(4):
                        nc.tensor.matmul(
                            pg[:], wih[:, j, 128 * m:128 * (m + 1)], xa[:, j, :],
                            start=(j == 0), stop=(j == 3),
                        )
                    pb = bpool.tile([128, 512], F32, tag="pb")
                    nc.vector.tensor_scalar(pb[:], pg[:], bias[:, m:m + 1], None, ADD)
                    nc.sync.dma_start(P_dram[:, m, t0:t0 + 512], pb[:])

            # ---- phase 2: LSTM recurrence ----
            with tc.For_i(0, S, BLK) as tb:
                nc.sync.dma_start(P_slot[:], P_dram[:, :, bass.ds(tb, BLK)])
                for s in range(BLK):
                    for m in range(16):
                        for j in range(4):
                            nc.tensor.matmul(
                                zp[:, m:m + 1], whh[:, j, 128 * m:128 * (m + 1)],
                                h_cur[:, j:j + 1], start=(j == 0), stop=(j == 3),
                            )
                    nc.vector.tensor_tensor(zs[:], zp[:], P_slot[:, :, s], ADD)
                    nc.scalar.activation(sig_if[:], zs[:, 0:8], Sig)
                    nc.scalar.activation(tanh_g[:], zs[:, 8:12], Tanh)
                    nc.scalar.activation(sig_o[:], zs[:, 12:16], Sig)
                    nc.vector.tensor_tensor(t2[:], sig_if[:, 0:4], tanh_g[:], MULT)
                    nc.vector.tensor_tensor(c_tmp[:], sig_if[:, 4:8], c_cur[:], MULT)
                    nc.vector.tensor_tensor(c_cur[:], c_tmp[:], t2[:], ADD)
                    nc.scalar.activation(tanh_c[:], c_cur[:], Tanh)
                    nc.vector.tensor_tensor(h_cur[:], sig_o[:], tanh_c[:], MULT)
                    nc.vector.tensor_copy(hs_blk[:, s, :], h_cur[:])
                nc.sync.dma_start(hs_dram[:, bass.ds(tb, BLK), :], hs_blk[:])

            # ---- phase 3: feats_partial.T [48, S] = Wout_half @ hs ----
            for ntile in range(S // 512):
                t0 = ntile * 512
                hc = xpool.tile([128, 512, 4], F32, tag="hc")
                nc.sync.dma_start(hc[:], hs_dram[:, t0:t0 + 512, :])
                fo = fpool.tile([48, 512], F32, tag="fo")
                for j in range(4):
                    nc.tensor.matmul(
                        fo[:], wout[:, j, :], hc[:, :, j],
                        start=(j == 0), stop=(j == 3),
                    )
                nc.vector.tensor_copy(feats_sb[:, t0:t0 + 512], fo[:])
            nc.sync.dma_start(feats_t[:], feats_sb[:])

    nc.compile()
    return nc


def _pm(v):  # [512] vector -> [128, 4] partition-major (idx = 128*j + p)
    return np.ascontiguousarray(np.asarray(v, np.float32).reshape(-1, 128).T)


def _core_inputs(xx, Wih, Whh, bih, bhh, hh, cc, wout_half):
    return {
        "x_t": np.ascontiguousarray(xx.T),
        "wih_t": np.ascontiguousarray(np.asarray(Wih, np.float32).T),
        "whh_t": np.ascontiguousarray(np.asarray(Whh, np.float32).T),
        "bias_pm": np.ascontiguousarray(
            (np.asarray(bih, np.float32) + np.asarray(bhh, np.float32))
            .reshape(16, 128).T),
        "h0_pm": _pm(hh),
        "c0_pm": _pm(cc),
        "wout_t": np.ascontiguousarray(np.asarray(wout_half, np.float32).T),
    }


def _in_maps(emb_table, Wih_f, Whh_f, bih_f, bhh_f, Wih_b, Whh_b, bih_b,
             bhh_b, W_out, h0, c0, sentence):
    emb_table = np.asarray(emb_table, np.float32)
    W_out = np.asarray(W_out, np.float32)
    h0 = np.asarray(h0, np.float32)
    c0 = np.asarray(c0, np.float32)
    idx = np.asarray(sentence).astype(np.int64)
    x = emb_table[idx]  # [S, E]
    in0 = _core_inputs(x, Wih_f, Whh_f, bih_f, bhh_f, h0[0, 0], c0[0, 0],
                       W_out[:, :H])
    in1 = _core_inputs(x[::-1], Wih_b, Whh_b, bih_b, bhh_b, h0[1, 0], c0[1, 0],
                       W_out[:, H:])
    return [in0, in1] + [in0] * 6


def kernel(emb_table, Wih_f, Whh_f, bih_f, bhh_f, Wih_b, Whh_b, bih_b,
           bhh_b, W_out, b_out, crf, h0, c0, sentence):
    b_out = np.asarray(b_out, np.float32)
    crf = np.asarray(crf, np.float32)
    maps = _in_maps(emb_table, Wih_f, Whh_f, bih_f, bhh_f, Wih_b, Whh_b,
                    bih_b, bhh_b, W_out, h0, c0, sentence)
    if "nc" not in _CACHE:
        _CACHE["nc"] = _build()
    res = run_bass_kernel_spmd(_CACHE["nc"], maps, core_ids=list(range(8)))
    f0 = np.asarray(res.results[0]["feats_t"], np.float32)
    f1 = np.asarray(res.results[1]["feats_t"], np.float32)
    feats = f0.T + f1.T[::-1] + b_out  # [S, T]

    # Viterbi (host): matches reference._viterbi
    sp = np.full(T, NEG, np.float32)
    sp[START] = 0.0
    steps = np.empty((S, T), np.int32)
    for t in range(S):
        score = sp[:, None] + feats[t][None, :] + crf
        steps[t] = np.argmax(score, axis=0)
        sp = np.max(score, axis=0).astype(np.float32)
    score_end = sp + crf[:, END]
    best_end = int(np.argmax(score_end))
    score_path = np.float32(score_end[best_end])
    labels = np.empty(S, np.int32)
    lbl = best_end
    for t in range(S - 1, -1, -1):
        labels[t] = lbl
        lbl = int(steps[t, lbl])
    return labels, score_path


def run_traced(inputs):
    """Re-run the cached kernel with NTFF tracing; returns exec_time_ns or None."""
    maps = _in_maps(inputs["emb_table"], inputs["Wih_f"], inputs["Whh_f"],
                    inputs["bih_f"], inputs["bhh_f"], inputs["Wih_b"],
                    inputs["Whh_b"], inputs["bih_b"], inputs["bhh_b"],
                    inputs["W_out"], inputs["h0"], inputs["c0"],
                    inputs["sentence"])
    if "nc" not in _CACHE:
        _CACHE["nc"] = _build()
    res = run_bass_kernel_spmd(_CACHE["nc"], maps, core_ids=list(range(8)),
                               trace=True)
    return res.exec_time_ns


# revision 6
# speedup vs baseline: 1.1343x; 1.1343x over previous
"""BiLSTM-CRF Trainium2 kernel.

Strategy: 2-way direction parallelism via SPMD (same program, different data).
Core 0 runs the forward LSTM on x; core 1 runs the backward LSTM on reversed x.
Each core: input-projection GEMM (P = Wih @ x + b) streamed to DRAM, sequential
LSTM recurrence with Whh stationary on the tensor engine (gates partition-major
[128,16]) consuming P in 64-step DMA blocks, then a partial feats GEMM
(hs @ Wout_half.T) returned transposed [48, 2048].
Host: embedding gather, feats combine, Viterbi scan + backtrace (tiny, serial).
"""
import sys

sys.path.insert(0, "/opt/trn_rl_repo")

import numpy as np
import concourse.bass as bass
import concourse.bacc as bacc
from concourse import mybir, tile
from concourse.bass_utils import run_bass_kernel_spmd

F32 = mybir.dt.float32
BF16 = mybir.dt.bfloat16
S, E, H, G, T = 2048, 512, 512, 2048, 48
BLK = 64
START, END = T - 2, T - 1
NEG = -10000.0

_CACHE = {}


def _build():
    nc = bacc.Bacc("TRN2", target_bir_lowering=False, debug=False)

    x_t = nc.dram_tensor("x_t", [E, S], F32, kind="ExternalInput")
    wih_t = nc.dram_tensor("wih_t", [E, G], F32, kind="ExternalInput")
    whh_t = nc.dram_tensor("whh_t", [H, G], BF16, kind="ExternalInput")
    bias_pm = nc.dram_tensor("bias_pm", [128, 16], F32, kind="ExternalInput")
    h0_pm = nc.dram_tensor("h0_pm", [128, 4], F32, kind="ExternalInput")
    c0_pm = nc.dram_tensor("c0_pm", [128, 4], F32, kind="ExternalInput")
    wout_t = nc.dram_tensor("wout_t", [H, T], F32, kind="ExternalInput")
    feats_t = nc.dram_tensor("feats_t", [T, S], F32, kind="ExternalOutput")
    # P[p, m, t] = proj[z=128m+p, t]; hs[p, t, j] = h_t[128j+p]
    P_dram = nc.dram_tensor("P_dram", [128, 16, S], F32)
    hs_dram = nc.dram_tensor("hs_dram", [128, S, 4], F32)

    Sig = mybir.ActivationFunctionType.Sigmoid
    Tanh = mybir.ActivationFunctionType.Tanh
    ADD = mybir.AluOpType.add
    MULT = mybir.AluOpType.mult

    with tile.TileContext(nc) as tc:
        with (
            tc.tile_pool(name="const", bufs=1) as cpool,
            tc.tile_pool(name="xin", bufs=2) as xpool,
            tc.tile_pool(name="bounce", bufs=4) as bpool,
            tc.tile_pool(name="state", bufs=1) as spool,
            tc.tile_pool(name="psz", bufs=1, space=bass.MemorySpace.PSUM) as zpool,
            tc.tile_pool(name="psg", bufs=2, space=bass.MemorySpace.PSUM) as gpool,
            tc.tile_pool(name="psf", bufs=2, space=bass.MemorySpace.PSUM) as fpool,
        ):
            wih = cpool.tile([128, 4, G], F32)   # [e%128, e//128, g]
            whh = cpool.tile([128, 4, G], BF16)   # [h%128, h//128, g]
            wout = cpool.tile([128, 4, T], F32)  # [h%128, h//128, tag]
            bias = cpool.tile([128, 16], F32)
            feats_sb = cpool.tile([48, S], F32)
            h_cur = spool.tile([128, 4], F32)
            h_bf = spool.tile([128, 4], BF16)
            c_cur = spool.tile([128, 4], F32)
            P_slot = spool.tile([128, 16, BLK], F32)
            hs_blk = spool.tile([128, BLK, 4], F32)
            zp = zpool.tile([128, 16], F32, tag="zp")
            zs = spool.tile([128, 16], F32)
            sig_ifo = spool.tile([128, 12], F32)
            tanh_g = spool.tile([128, 4], F32)
            sig_o = spool.tile([128, 4], F32)
            t2 = spool.tile([128, 4], F32)
            c_tmp = spool.tile([128, 4], F32)
            tanh_c = spool.tile([128, 4], F32)

            for j in range(4):
                nc.sync.dma_start(wih[:, j, :], wih_t[128 * j:128 * (j + 1), :])
                nc.sync.dma_start(whh[:, j, :], whh_t[128 * j:128 * (j + 1), :])
                nc.sync.dma_start(wout[:, j, :], wout_t[128 * j:128 * (j + 1), :])
            nc.sync.dma_start(bias[:], bias_pm[:])
            nc.sync.dma_start(h_cur[:], h0_pm[:])
            nc.sync.dma_start(c_cur[:], c0_pm[:])
            nc.vector.tensor_copy(h_bf[:], h_cur[:])

            # ---- phase 1: P = Wih @ x + b  -> P_dram ----
            for ntile in range(S // 512):
                t0 = ntile * 512
                xa = xpool.tile([128, 4, 512], F32, tag="xa")
                for j in range(4):
                    nc.sync.dma_start(xa[:, j, :], x_t[128 * j:128 * (j + 1), t0:t0 + 512])
                for m in range(16):
                    pg = gpool.tile([128, 512], F32, tag="pg")
                    for j in range(4):
                        nc.tensor.matmul(
                            pg[:], wih[:, j, 128 * m:128 * (m + 1)], xa[:, j, :],
                            start=(j == 0), stop=(j == 3),
                        )
                    pb = bpool.tile([128, 512], F32, tag="pb")
                    nc.vector.tensor_scalar(pb[:], pg[:], bias[:, m:m + 1], None, ADD)
                    nc.sync.dma_start(P_dram[:, m, t0:t0 + 512], pb[:])

            # ---- phase 2: LSTM recurrence ----
            with tc.For_i(0, S, BLK) as tb:
                nc.sync.dma_start(P_slot[:], P_dram[:, :, bass.ds(tb, BLK)])
                for s in range(BLK):
                    for m in range(16):
                        for j in range(4):
                            nc.tensor.matmul(
                                zp[:, m:m + 1], whh[:, j, 128 * m:128 * (m + 1)],
                                h_bf[:, j:j + 1], start=(j == 0), stop=(j == 3),
                            )
                    nc.vector.tensor_tensor(zs[:], zp[:], P_slot[:, :, s], ADD)
                    nc.scalar.activation(sig_ifo[:], zs[:, 0:12], Sig)
                    nc.scalar.activation(tanh_g[:], zs[:, 12:16], Tanh)
                    nc.vector.tensor_tensor(t2[:], sig_ifo[:, 0:4], tanh_g[:], MULT)
                    nc.vector.tensor_tensor(c_tmp[:], sig_ifo[:, 4:8], c_cur[:], MULT)
                    nc.vector.tensor_tensor(c_cur[:], c_tmp[:], t2[:], ADD)
                    nc.scalar.activation(tanh_c[:], c_cur[:], Tanh)
                    nc.vector.tensor_tensor(h_bf[:], sig_ifo[:, 8:12], tanh_c[:], MULT)
                    nc.vector.tensor_tensor(h_cur[:], sig_ifo[:, 8:12], tanh_c[:], MULT)
                    nc.vector.tensor_copy(hs_blk[:, s, :], h_cur[:])
                nc.sync.dma_start(hs_dram[:, bass.ds(tb, BLK), :], hs_blk[:])

            # ---- phase 3: feats_partial.T [48, S] = Wout_half @ hs ----
            for ntile in range(S // 512):
                t0 = ntile * 512
                hc = xpool.tile([128, 512, 4], F32, tag="hc")
                nc.sync.dma_start(hc[:], hs_dram[:, t0:t0 + 512, :])
                fo = fpool.tile([48, 512], F32, tag="fo")
                for j in range(4):
                    nc.tensor.matmul(
                        fo[:], wout[:, j, :], hc[:, :, j],
                        start=(j == 0), stop=(j == 3),
                    )
                nc.vector.tensor_copy(feats_sb[:, t0:t0 + 512], fo[:])
            nc.sync.dma_start(feats_t[:], feats_sb[:])

    nc.compile()
    return nc


def _pm(v):  # [512] vector -> [128, 4] partition-major (idx = 128*j + p)
    return np.ascontiguousarray(np.asarray(v, np.float32).reshape(-1, 128).T)


_PERM = np.r_[0:1024, 1536:2048, 1024:1536]  # gate order i,f,g,o -> i,f,o,g


def _core_inputs(xx, Wih, Whh, bih, bhh, hh, cc, wout_half):
    import ml_dtypes
    return {
        "x_t": np.ascontiguousarray(xx.T),
        "wih_t": np.ascontiguousarray(np.asarray(Wih, np.float32)[_PERM].T),
        "whh_t": np.ascontiguousarray(
            np.asarray(Whh, np.float32)[_PERM].T.astype(ml_dtypes.bfloat16)),
        "bias_pm": np.ascontiguousarray(
            (np.asarray(bih, np.float32) + np.asarray(bhh, np.float32))[_PERM]
            .reshape(16, 128).T),
        "h0_pm": _pm(hh),
        "c0_pm": _pm(cc),
        "wout_t": np.ascontiguousarray(np.asarray(wout_half, np.float32).T),
    }


def _in_maps(emb_table, Wih_f, Whh_f, bih_f, bhh_f, Wih_b, Whh_b, bih_b,
             bhh_b, W_out, h0, c0, sentence):
    emb_table = np.asarray(emb_table, np.float32)
    W_out = np.asarray(W_out, np.float32)
    h0 = np.asarray(h0, np.float32)
    c0 = np.asarray(c0, np.float32)
    idx = np.asarray(sentence).astype(np.int64)
    x = emb_table[idx]  # [S, E]
    in0 = _core_inputs(x, Wih_f, Whh_f, bih_f, bhh_f, h0[0, 0], c0[0, 0],
                       W_out[:, :H])
    in1 = _core_inputs(x[::-1], Wih_b, Whh_b, bih_b, bhh_b, h0[1, 0], c0[1, 0],
                       W_out[:, H:])
    return [in0, in1] + [in0] * 6


def kernel(emb_table, Wih_f, Whh_f, bih_f, bhh_f, Wih_b, Whh_b, bih_b,
           bhh_b, W_out, b_out, crf, h0, c0, sentence):
    b_out = np.asarray(b_out, np.float32)
    crf = np.asarray(crf, np.float32)
    maps = _in_maps(emb_table, Wih_f, Whh_f, bih_f, bhh_f, Wih_b, Whh_b,
                    bih_b, bhh_b, W_out, h0, c0, sentence)
    if "nc" not in _CACHE:
        _CACHE["nc"] = _build()
    res = run_bass_kernel_spmd(_CACHE["nc"], maps, core_ids=list(range(8)))
    f0 = np.asarray(res.results[0]["feats_t"], np.float32)
    f1 = np.asarray(res.results[1]["feats_t"], np.float32)
    feats = f0.T + f1.T[::-1] + b_out  # [S, T]

    # Viterbi (host): matches reference._viterbi
    sp = np.full(T, NEG, np.float32)
    sp[START] = 0.0
    steps = np.empty((S, T), np.int32)
    for t in range(S):
        score = sp[:, None] + feats[t][None, :] + crf
        steps[t] = np.argmax(score, axis=0)
        sp = np.max(score, axis=0).astype(np.float32)
    score_end = sp + crf[:, END]
    best_end = int(np.argmax(score_end))
    score_path = np.float32(score_end[best_end])
    labels = np.empty(S, np.int32)
    lbl = best_end
    for t in range(S - 1, -1, -1):
        labels[t] = lbl
        lbl = int(steps[t, lbl])
    return labels, score_path


def run_traced(inputs):
    """Re-run the cached kernel with NTFF tracing; returns exec_time_ns or None."""
    maps = _in_maps(inputs["emb_table"], inputs["Wih_f"], inputs["Whh_f"],
                    inputs["bih_f"], inputs["bhh_f"], inputs["Wih_b"],
                    inputs["Whh_b"], inputs["bih_b"], inputs["bhh_b"],
                    inputs["W_out"], inputs["h0"], inputs["c0"],
                    inputs["sentence"])
    if "nc" not in _CACHE:
        _CACHE["nc"] = _build()
    res = run_bass_kernel_spmd(_CACHE["nc"], maps, core_ids=list(range(8)),
                               trace=True)
    return res.exec_time_ns


# revision 7
# speedup vs baseline: 1.1456x; 1.0100x over previous
"""BiLSTM-CRF Trainium2 kernel.

Strategy: 2-way direction parallelism via SPMD (same program, different data).
Core 0 runs the forward LSTM on x; core 1 runs the backward LSTM on reversed x.
Each core: input-projection GEMM (P = Wih @ x + b) streamed to DRAM, sequential
LSTM recurrence with Whh stationary on the tensor engine (gates partition-major
[128,16]) consuming P in 64-step DMA blocks, then a partial feats GEMM
(hs @ Wout_half.T) returned transposed [48, 2048].
Host: embedding gather, feats combine, Viterbi scan + backtrace (tiny, serial).
"""
import sys

sys.path.insert(0, "/opt/trn_rl_repo")

import numpy as np
import concourse.bass as bass
import concourse.bacc as bacc
from concourse import mybir, tile
from concourse.bass_utils import run_bass_kernel_spmd

F32 = mybir.dt.float32
BF16 = mybir.dt.bfloat16
S, E, H, G, T = 2048, 512, 512, 2048, 48
BLK = 64
START, END = T - 2, T - 1
NEG = -10000.0

_CACHE = {}


def _build():
    nc = bacc.Bacc("TRN2", target_bir_lowering=False, debug=False)

    x_t = nc.dram_tensor("x_t", [E, S], F32, kind="ExternalInput")
    wih_t = nc.dram_tensor("wih_t", [E, G], F32, kind="ExternalInput")
    whh_t = nc.dram_tensor("whh_t", [H, G], BF16, kind="ExternalInput")
    bias_pm = nc.dram_tensor("bias_pm", [128, 16], F32, kind="ExternalInput")
    h0_pm = nc.dram_tensor("h0_pm", [128, 4], F32, kind="ExternalInput")
    c0_pm = nc.dram_tensor("c0_pm", [128, 4], F32, kind="ExternalInput")
    wout_t = nc.dram_tensor("wout_t", [H, T], F32, kind="ExternalInput")
    feats_t = nc.dram_tensor("feats_t", [T, S], F32, kind="ExternalOutput")
    # P[p, m, t] = proj[z=128m+p, t]; hs[p, t, j] = h_t[128j+p]
    P_dram = nc.dram_tensor("P_dram", [128, 16, S], F32)
    hs_dram = nc.dram_tensor("hs_dram", [128, S, 4], F32)

    Sig = mybir.ActivationFunctionType.Sigmoid
    Tanh = mybir.ActivationFunctionType.Tanh
    ADD = mybir.AluOpType.add
    MULT = mybir.AluOpType.mult

    with tile.TileContext(nc) as tc:
        with (
            tc.tile_pool(name="const", bufs=1) as cpool,
            tc.tile_pool(name="xin", bufs=2) as xpool,
            tc.tile_pool(name="bounce", bufs=4) as bpool,
            tc.tile_pool(name="state", bufs=1) as spool,
            tc.tile_pool(name="psz", bufs=1, space=bass.MemorySpace.PSUM) as zpool,
            tc.tile_pool(name="psg", bufs=2, space=bass.MemorySpace.PSUM) as gpool,
            tc.tile_pool(name="psf", bufs=2, space=bass.MemorySpace.PSUM) as fpool,
        ):
            wih = cpool.tile([128, 4, G], F32)   # [e%128, e//128, g]
            whh = cpool.tile([128, 4, G], BF16)   # [h%128, h//128, g]
            wout = cpool.tile([128, 4, T], F32)  # [h%128, h//128, tag]
            bias = cpool.tile([128, 16], F32)
            feats_sb = cpool.tile([48, S], F32)
            h_cur = spool.tile([128, 4], F32)
            h_bf = spool.tile([128, 4], BF16)
            c_cur = spool.tile([128, 4], F32)
            P_slot = spool.tile([128, 16, BLK], F32)
            hs_blk = spool.tile([128, BLK, 4], F32)
            zp = zpool.tile([128, 16], F32, tag="zp")
            zs = spool.tile([128, 16], F32)
            sig_ifo = spool.tile([128, 12], F32)
            tanh_g = spool.tile([128, 4], F32)
            sig_o = spool.tile([128, 4], F32)
            t2 = spool.tile([128, 4], F32)
            c_tmp = spool.tile([128, 4], F32)
            tanh_c = spool.tile([128, 4], F32)

            for j in range(4):
                nc.sync.dma_start(wih[:, j, :], wih_t[128 * j:128 * (j + 1), :])
                nc.sync.dma_start(whh[:, j, :], whh_t[128 * j:128 * (j + 1), :])
                nc.sync.dma_start(wout[:, j, :], wout_t[128 * j:128 * (j + 1), :])
            nc.sync.dma_start(bias[:], bias_pm[:])
            nc.sync.dma_start(h_cur[:], h0_pm[:])
            nc.sync.dma_start(c_cur[:], c0_pm[:])
            nc.vector.tensor_copy(h_bf[:], h_cur[:])

            # ---- phase 1: P = Wih @ x + b  -> P_dram ----
            for ntile in range(S // 512):
                t0 = ntile * 512
                xa = xpool.tile([128, 4, 512], F32, tag="xa")
                for j in range(4):
                    nc.sync.dma_start(xa[:, j, :], x_t[128 * j:128 * (j + 1), t0:t0 + 512])
                for m in range(16):
                    pg = gpool.tile([128, 512], F32, tag="pg")
                    for j in range(4):
                        nc.tensor.matmul(
                            pg[:], wih[:, j, 128 * m:128 * (m + 1)], xa[:, j, :],
                            start=(j == 0), stop=(j == 3),
                        )
                    pb = bpool.tile([128, 512], F32, tag="pb")
                    nc.vector.tensor_scalar(pb[:], pg[:], bias[:, m:m + 1], None, ADD)
                    nc.sync.dma_start(P_dram[:, m, t0:t0 + 512], pb[:])

            # ---- phase 2: LSTM recurrence ----
            with tc.For_i(0, S, BLK) as tb:
                nc.sync.dma_start(P_slot[:], P_dram[:, :, bass.ds(tb, BLK)])
                for s in range(BLK):
                    for m in range(16):
                        for j in range(4):
                            nc.tensor.matmul(
                                zp[:, m:m + 1], whh[:, j, 128 * m:128 * (m + 1)],
                                h_bf[:, j:j + 1], start=(j == 0), stop=(j == 3),
                            )
                    nc.vector.tensor_tensor(zs[:], zp[:], P_slot[:, :, s], ADD)
                    nc.scalar.activation(sig_ifo[:], zs[:, 0:12], Sig)
                    nc.scalar.activation(tanh_g[:], zs[:, 12:16], Tanh)
                    nc.vector.tensor_tensor(t2[:], sig_ifo[:, 0:4], tanh_g[:], MULT)
                    nc.vector.tensor_tensor(c_tmp[:], sig_ifo[:, 4:8], c_cur[:], MULT)
                    nc.vector.tensor_tensor(c_cur[:], c_tmp[:], t2[:], ADD)
                    nc.scalar.activation(tanh_c[:], c_cur[:], Tanh)
                    nc.vector.tensor_tensor(h_bf[:], sig_ifo[:, 8:12], tanh_c[:], MULT)
                    nc.vector.tensor_tensor(hs_blk[:, s, :], sig_ifo[:, 8:12], tanh_c[:], MULT)
                nc.sync.dma_start(hs_dram[:, bass.ds(tb, BLK), :], hs_blk[:])

            # ---- phase 3: feats_partial.T [48, S] = Wout_half @ hs ----
            for ntile in range(S // 512):
                t0 = ntile * 512
                hc = xpool.tile([128, 512, 4], F32, tag="hc")
                nc.sync.dma_start(hc[:], hs_dram[:, t0:t0 + 512, :])
                fo = fpool.tile([48, 512], F32, tag="fo")
                for j in range(4):
                    nc.tensor.matmul(
                        fo[:], wout[:, j, :], hc[:, :, j],
                        start=(j == 0), stop=(j == 3),
                    )
                nc.vector.tensor_copy(feats_sb[:, t0:t0 + 512], fo[:])
            nc.sync.dma_start(feats_t[:], feats_sb[:])

    nc.compile()
    return nc


def _pm(v):  # [512] vector -> [128, 4] partition-major (idx = 128*j + p)
    return np.ascontiguousarray(np.asarray(v, np.float32).reshape(-1, 128).T)


_PERM = np.r_[0:1024, 1536:2048, 1024:1536]  # gate order i,f,g,o -> i,f,o,g


def _core_inputs(xx, Wih, Whh, bih, bhh, hh, cc, wout_half):
    import ml_dtypes
    return {
        "x_t": np.ascontiguousarray(xx.T),
        "wih_t": np.ascontiguousarray(np.asarray(Wih, np.float32)[_PERM].T),
        "whh_t": np.ascontiguousarray(
            np.asarray(Whh, np.float32)[_PERM].T.astype(ml_dtypes.bfloat16)),
        "bias_pm": np.ascontiguousarray(
            (np.asarray(bih, np.float32) + np.asarray(bhh, np.float32))[_PERM]
            .reshape(16, 128).T),
        "h0_pm": _pm(hh),
        "c0_pm": _pm(cc),
        "wout_t": np.ascontiguousarray(np.asarray(wout_half, np.float32).T),
    }


def _in_maps(emb_table, Wih_f, Whh_f, bih_f, bhh_f, Wih_b, Whh_b, bih_b,
             bhh_b, W_out, h0, c0, sentence):
    emb_table = np.asarray(emb_table, np.float32)
    W_out = np.asarray(W_out, np.float32)
    h0 = np.asarray(h0, np.float32)
    c0 = np.asarray(c0, np.float32)
    idx = np.asarray(sentence).astype(np.int64)
    x = emb_table[idx]  # [S, E]
    in0 = _core_inputs(x, Wih_f, Whh_f, bih_f, bhh_f, h0[0, 0], c0[0, 0],
                       W_out[:, :H])
    in1 = _core_inputs(x[::-1], Wih_b, Whh_b, bih_b, bhh_b, h0[1, 0], c0[1, 0],
                       W_out[:, H:])
    return [in0, in1] + [in0] * 6


def kernel(emb_table, Wih_f, Whh_f, bih_f, bhh_f, Wih_b, Whh_b, bih_b,
           bhh_b, W_out, b_out, crf, h0, c0, sentence):
    b_out = np.asarray(b_out, np.float32)
    crf = np.asarray(crf, np.float32)
    maps = _in_maps(emb_table, Wih_f, Whh_f, bih_f, bhh_f, Wih_b, Whh_b,
                    bih_b, bhh_b, W_out, h0, c0, sentence)
    if "nc" not in _CACHE:
        _CACHE["nc"] = _build()
    res = run_bass_kernel_spmd(_CACHE["nc"], maps, core_ids=list(range(8)))
    f0 = np.asarray(res.results[0]["feats_t"], np.float32)
    f1 = np.asarray(res.results[1]["feats_t"], np.float32)
    feats = f0.T + f1.T[::-1] + b_out  # [S, T]

    # Viterbi (host): matches reference._viterbi
    sp = np.full(T, NEG, np.float32)
    sp[START] = 0.0
    steps = np.empty((S, T), np.int32)
    for t in range(S):
        score = sp[:, None] + feats[t][None, :] + crf
        steps[t] = np.argmax(score, axis=0)
        sp = np.max(score, axis=0).astype(np.float32)
    score_end = sp + crf[:, END]
    best_end = int(np.argmax(score_end))
    score_path = np.float32(score_end[best_end])
    labels = np.empty(S, np.int32)
    lbl = best_end
    for t in range(S - 1, -1, -1):
        labels[t] = lbl
        lbl = int(steps[t, lbl])
    return labels, score_path


def run_traced(inputs):
    """Re-run the cached kernel with NTFF tracing; returns exec_time_ns or None."""
    maps = _in_maps(inputs["emb_table"], inputs["Wih_f"], inputs["Whh_f"],
                    inputs["bih_f"], inputs["bhh_f"], inputs["Wih_b"],
                    inputs["Whh_b"], inputs["bih_b"], inputs["bhh_b"],
                    inputs["W_out"], inputs["h0"], inputs["c0"],
                    inputs["sentence"])
    if "nc" not in _CACHE:
        _CACHE["nc"] = _build()
    res = run_bass_kernel_spmd(_CACHE["nc"], maps, core_ids=list(range(8)),
                               trace=True)
    return res.exec_time_ns
